# revision 1
# baseline (speedup 1.0000x reference)
"""Trainium2 Bass kernel: bilinear interpolation from BEV feature maps.

reference semantics (interpolate_from_bev_features, correction=False):
  keypoints (B, N, 3) f32; bev_features (B, C, H, W) f32; bev_stride scalar
  out (B, N, C) f32: bilinear sample at x = kp_x/(0.05*stride),
  y = (kp_y+40)/(0.05*stride); corner indices clamped to [0, 187]; weights
  from clamped corner coords (out-of-range y cancels to exactly 0).

Sharding: 8 cores = batch (4) x channel-half (2).

Per-core plan (SBUF gather ucode measured ~20 ns/element on this part, so
the gather runs on the DMA engines instead):
  Phase A: stream-transpose the (128ch, H*W_PACK) slab into a DRAM scratch
    TBEV[px, 128ch]: DMA load [128, 2048] -> DVE 32x32 stream-transpose ->
    4 DMA stores with block-permuted 3D access patterns (128B runs).
  Phase B: dma_gather (MoE-style SWDGE gather) fetches, per keypoint corner
    row, a 384-element run (3 pixels x 128ch starting at the even pixel
    below x0) out of an overlapped [V, 384]/stride-256 view of TBEV.
    int16 gather indices address 256-element pair rows (max 16731).
    The x-parity selects which 2 of the 3 pixels matter - folded into
    per-keypoint 3-slot weights, applied on DVE via stride-0 broadcast.
    Output lands keypoint-major: straight DMA out.

Shapes hardcoded per problem spec: B=4 N=4096 C=256 H=W=188 (x<=176 so
only W_PACK=178 columns are ever addressed).
"""
import os
import sys

for _p in ('/opt/trn_rl_repo', '/root/.axon_site/_ro/trn_rl_repo'):
    if os.path.isdir(_p) and _p not in sys.path:
        sys.path.append(_p)

import numpy as np

B, N, C, H, W = 4, 4096, 256, 188, 188
W_PACK = 178                  # x <= 176 -> x1 <= 177; cols 178..187 never read
FLAT = H * W_PACK             # 33464 pixels
NLOAD = 17                    # phase-A loads of [128, 2048]
FLAT_PAD = NLOAD * 2048       # 34816
VPAIR = FLAT_PAD * 128 // 256 - 1   # overlapped 384-elem rows, stride 256
BCH = 512                     # gather indices per dma_gather call
NBCH = N // BCH               # 4 phase-B chunks
GPC = BCH // 128              # keypoint blocks of 128 per chunk (8)
N_CORES = 8

_compiled = {}


def _build(scale: float, ybias: float, debug_taps: bool = False):
    import concourse.bacc as bacc
    import concourse.mybir as mybir
    import concourse.tile as tile
    import contextlib
    from concourse.bass import AP

    dt = mybir.dt
    nc = bacc.Bacc("TRN2", target_bir_lowering=False, debug=False,
                   num_devices=N_CORES)

    slab_d = nc.dram_tensor("slab", [128, FLAT_PAD], dt.float32, kind="ExternalInput")
    kp_d = nc.dram_tensor("kp", [N, 3], dt.float32, kind="ExternalInput")
    out_d = nc.dram_tensor("out", [N, 128], dt.float32, kind="ExternalOutput")
    tbev_d = nc.dram_tensor("tbev", [FLAT_PAD, 128], dt.float32)

    taps = {}
    if debug_taps:
        for nm, shp in [("t_X0", [128, 32]), ("t_QX", [128, 32]),
                        ("t_W3AC", [128, 96]), ("t_I0w", [128, 256]),
                        ("t_TB", [128, 128])]:
            taps[nm] = nc.dram_tensor(nm, shp, dt.float32, kind="ExternalOutput")

    # keypoint n = s*128 + p  (block-major) for weights;
    # n = s*16 + r (wrapped-16) for gather indices
    kp_blk = kp_d.ap().rearrange("(s p) c -> p s c", p=128)        # [128, 32, 3]
    kp_wrp = kp_d.ap().rearrange("(s r) c -> r s c", r=16)         # [16, 256, 3]
    out_r = out_d.ap().rearrange("(cb g p) c -> cb p g c", g=GPC, p=128)

    AF = mybir.ActivationFunctionType
    OP = mybir.AluOpType

    with tile.TileContext(nc) as tc, contextlib.ExitStack() as ctx:
        lda = ctx.enter_context(tc.tile_pool(name="lda", bufs=4))
        tta = ctx.enter_context(tc.tile_pool(name="tta", bufs=4))
        meta = ctx.enter_context(tc.tile_pool(name="meta", bufs=1))
        gat = ctx.enter_context(tc.tile_pool(name="gat", bufs=2))
        blend = ctx.enter_context(tc.tile_pool(name="blend", bufs=2))
        dram = ctx.enter_context(tc.tile_pool(name="dram", bufs=1, space="DRAM"))
        TB = dram.tile([FLAT_PAD, 128], dt.float32)

        # ---- phase A: slab[c, px] -> TBEV[px, c] ----
        # StreamTranspose: TTB[32a+r, 32m+s] = BLK[32a+s, 32m+r]; one store
        # per 32-channel group a keeps both DMA access patterns at 3 dims.
        for kb in range(NLOAD):
            BLK = lda.tile([128, 2048], dt.float32, tag="BLK")
            nc.sync.dma_start(BLK[:], slab_d.ap()[:, kb * 2048:(kb + 1) * 2048])
            TTB = tta.tile([128, 2048], dt.float32, tag="TTB")
            nc.vector.transpose(TTB[:], BLK[:])
            for a in range(4):
                dst = TB[kb * 2048:(kb + 1) * 2048, a * 32:(a + 1) * 32] \
                    .rearrange("(m r) s -> r m s", r=32)
                src = TTB[a * 32:(a + 1) * 32, :].rearrange("p (m s) -> p m s", s=32)
                eng = (nc.sync, nc.scalar, nc.sync, nc.scalar)[a]
                eng.dma_start(dst, src)

        # ---- keypoint math ----
        def floor_of(v_ap, pool, nfree, tag):
            """floor(v) for v >= 0, exact under trunc or round f32<->i32."""
            CI = pool.tile([128, nfree], dt.int32, tag=tag + "i")
            nc.vector.tensor_copy(out=CI[:], in_=v_ap)
            CF = pool.tile([128, nfree], dt.float32, tag=tag + "f")
            nc.vector.tensor_copy(out=CF[:], in_=CI[:])
            GT = pool.tile([128, nfree], dt.float32, tag=tag + "g")
            nc.vector.tensor_tensor(GT[:], CF[:], v_ap, op=OP.is_gt)
            OUT = pool.tile([128, nfree], dt.float32, tag=tag + "o")
            nc.vector.tensor_tensor(OUT[:], CF[:], GT[:], op=OP.subtract)
            return OUT

        def coords(x_ap, y_ap, nfree, pfx):
            """-> (XS, YS, X0, Y0, Y1) f32 [128, nfree], reference clamps."""
            XS = meta.tile([128, nfree], dt.float32, tag=pfx + "XS")
            nc.scalar.activation(XS[:], x_ap, AF.Copy, bias=0.0, scale=scale)
            YS = meta.tile([128, nfree], dt.float32, tag=pfx + "YS")
            nc.scalar.activation(YS[:], y_ap, AF.Copy, bias=ybias, scale=scale)
            X0 = floor_of(XS[:], meta, nfree, pfx + "fx")
            T = floor_of(YS[:], meta, nfree, pfx + "fy")
            Y0 = meta.tile([128, nfree], dt.float32, tag=pfx + "Y0")
            nc.vector.tensor_scalar(Y0[:], T[:], float(H - 1), None, OP.min)
            Y1 = meta.tile([128, nfree], dt.float32, tag=pfx + "Y1")
            nc.vector.tensor_scalar(Y1[:], T[:], 1.0, float(H - 1), OP.add, OP.min)
            return XS, YS, X0, Y0, Y1

        # block-major pipeline: weights
        KP = meta.tile([128, 96], dt.float32)
        kp3 = KP[:].rearrange("p (s c) -> p s c", c=3)
        nc.sync.dma_start(kp3, kp_blk)
        XS, YS, X0, Y0, Y1 = coords(kp3[:, :, 0], kp3[:, :, 1], 32, "n")

        FX = meta.tile([128, 32], dt.float32)
        nc.vector.tensor_tensor(FX[:], XS[:], X0[:], op=OP.subtract)
        WXL = meta.tile([128, 32], dt.float32)
        nc.vector.tensor_scalar(WXL[:], FX[:], 1.0, -1.0, OP.subtract, OP.mult)
        WY0 = meta.tile([128, 32], dt.float32)
        nc.vector.tensor_tensor(WY0[:], Y1[:], YS[:], op=OP.subtract)
        WY1 = meta.tile([128, 32], dt.float32)
        nc.vector.tensor_tensor(WY1[:], YS[:], Y0[:], op=OP.subtract)
        # x parity qx = x0 mod 2 (row base y*178 is even)
        XH = meta.tile([128, 32], dt.float32)
        nc.vector.tensor_scalar(XH[:], X0[:], 0.5, None, OP.mult)
        XHF = floor_of(XH[:], meta, 32, "nqh")
        QX = meta.tile([128, 32], dt.float32)
        nc.vector.tensor_scalar(QX[:], XHF[:], -2.0, None, OP.mult)
        nc.vector.tensor_tensor(QX[:], X0[:], QX[:], op=OP.add)
        QM = meta.tile([128, 32], dt.float32)
        nc.vector.tensor_scalar(QM[:], QX[:], 1.0, -1.0, OP.subtract, OP.mult)
        # 3-slot x weights: u0 = wxl*(1-qx); u1 = wxl*qx + fx*(1-qx); u2 = fx*qx
        U0 = meta.tile([128, 32], dt.float32)
        nc.vector.tensor_tensor(U0[:], WXL[:], QM[:], op=OP.mult)
        U1 = meta.tile([128, 32], dt.float32)
        T1 = meta.tile([128, 32], dt.float32)
        nc.vector.tensor_tensor(T1[:], WXL[:], QX[:], op=OP.mult)
        nc.vector.tensor_tensor(U1[:], FX[:], QM[:], op=OP.mult)
        nc.vector.tensor_tensor(U1[:], U1[:], T1[:], op=OP.add)
        U2 = meta.tile([128, 32], dt.float32)
        nc.vector.tensor_tensor(U2[:], FX[:], QX[:], op=OP.mult)
        W3AC = meta.tile([128, 32, 3], dt.float32)
        W3BD = meta.tile([128, 32, 3], dt.float32)
        for k, u in enumerate((U0, U1, U2)):
            nc.vector.tensor_tensor(W3AC[:, :, k], u[:], WY0[:], op=OP.mult)
            nc.vector.tensor_tensor(W3BD[:, :, k], u[:], WY1[:], op=OP.mult)

        # wrapped-16 pipeline: gather pair-row indices (int16)
        KPW = meta.tile([128, 768], dt.float32)
        kpw3 = KPW[:].rearrange("p (s c) -> p s c", c=3)
        for g in range(8):
            nc.sync.dma_start(kpw3[g * 16:(g + 1) * 16], kp_wrp)
        _, _, X0w, Y0w, Y1w = coords(kpw3[:, :, 0], kpw3[:, :, 1], 256, "w")
        IDXW = []
        for nm, yy in (("I0", Y0w), ("I1", Y1w)):
            base = meta.tile([128, 256], dt.float32, tag=nm + "b")
            nc.vector.tensor_scalar(base[:], yy[:], float(W_PACK), None, OP.mult)
            nc.vector.tensor_tensor(base[:], base[:], X0w[:], op=OP.add)
            nc.vector.tensor_scalar(base[:], base[:], 0.5, None, OP.mult)
            bf = floor_of(base[:], meta, 256, nm + "fh")
            ii = meta.tile([128, 256], dt.int16, tag=nm + "w")
            nc.vector.tensor_copy(out=ii[:], in_=bf[:])
            IDXW.append(ii)
        I0W, I1W = IDXW

        if debug_taps:
            nc.sync.dma_start(taps["t_X0"].ap(), X0[:])
            nc.sync.dma_start(taps["t_QX"].ap(), QX[:])
            nc.sync.dma_start(taps["t_W3AC"].ap(),
                              W3AC[:].rearrange("p s c -> p (s c)"))
            I0f = meta.tile([128, 256], dt.float32)
            nc.vector.tensor_copy(out=I0f[:], in_=I0W[:])
            nc.sync.dma_start(taps["t_I0w"].ap(), I0f[:])
            nc.sync.dma_start(taps["t_TB"].ap(), TB[0:128, :])

        # overlapped pair-row view of TBEV: row v = elements [v*256, v*256+384)
        tb_pairs = AP(TB[:].tensor, TB[:].offset, [[256, VPAIR], [1, 384]])

        # ---- phase B: gather + in-place blend + store ----
        for cb in range(NBCH):
            wsl = slice(cb * (BCH // 16), (cb + 1) * (BCH // 16))
            bsl = slice(cb * GPC, (cb + 1) * GPC)
            G0 = gat.tile([128, GPC, 3, 128], dt.float32, tag="G0")
            nc.gpsimd.dma_gather(
                out_ap=G0[:].rearrange("p g t c -> p g (t c)"),
                in_ap=tb_pairs, idxs_ap=I0W[:, wsl],
                num_idxs=BCH, num_idxs_reg=BCH, elem_size=384, elem_step=256)
            G1 = gat.tile([128, GPC, 3, 128], dt.float32, tag="G1")
            nc.gpsimd.dma_gather(
                out_ap=G1[:].rearrange("p g t c -> p g (t c)"),
                in_ap=tb_pairs, idxs_ap=I1W[:, wsl],
                num_idxs=BCH, num_idxs_reg=BCH, elem_size=384, elem_step=256)

            P0 = blend.tile([128, GPC, 3, 128], dt.float32, tag="P0")
            w3ac_b = W3AC[:, bsl, :, None].to_broadcast((128, GPC, 3, 128))
            nc.vector.tensor_tensor(P0[:], G0[:], w3ac_b, op=OP.mult)
            P1 = blend.tile([128, GPC, 3, 128], dt.float32, tag="P1")
            w3bd_b = W3BD[:, bsl, :, None].to_broadcast((128, GPC, 3, 128))
            nc.vector.tensor_tensor(P1[:], G1[:], w3bd_b, op=OP.mult)
            S = blend.tile([128, GPC, 3, 128], dt.float32, tag="S")
            nc.vector.tensor_tensor(S[:], P0[:], P1[:], op=OP.add)
            OUTG = blend.tile([128, GPC, 128], dt.float32, tag="OUTG")
            nc.vector.tensor_tensor(OUTG[:], S[:, :, 0, :], S[:, :, 1, :], op=OP.add)
            nc.vector.tensor_tensor(OUTG[:], OUTG[:], S[:, :, 2, :], op=OP.add)
            nc.sync.dma_start(out_r[cb], OUTG[:])

    nc.compile()
    return nc


def _get(scale: float, ybias: float):
    key = (round(scale, 9), round(ybias, 9))
    if key not in _compiled:
        _compiled[key] = _build(scale, ybias)
    return _compiled[key]


def _prepare_in_maps(keypoints: np.ndarray, bev_features: np.ndarray):
    kp = np.ascontiguousarray(keypoints, dtype=np.float32)
    bev = np.asarray(bev_features, dtype=np.float32)
    in_maps = []
    for core in range(N_CORES):
        b, ch = core // 2, core % 2
        sl = slice(ch * 128, (ch + 1) * 128)
        slab = np.zeros((128, FLAT_PAD), dtype=np.float32)
        slab[:, :FLAT] = bev[b, sl, :, :W_PACK].reshape(128, FLAT)
        in_maps.append({"slab": slab, "kp": kp[b]})
    return in_maps


def _assemble(results) -> np.ndarray:
    out = np.empty((B, N, C), dtype=np.float32)
    for core in range(N_CORES):
        b, ch = core // 2, core % 2
        out[b, :, ch * 128:(ch + 1) * 128] = np.asarray(results[core]["out"])
    return out


def _scale_bias(bev_stride):
    stride = float(np.asarray(bev_stride))
    scale = 1.0 / (0.05 * stride)
    return scale, 40.0 * scale


def kernel(keypoints: np.ndarray, bev_features: np.ndarray, bev_stride) -> np.ndarray:
    from concourse.bass_utils import run_bass_kernel_spmd

    scale, ybias = _scale_bias(bev_stride)
    nc = _get(scale, ybias)
    in_maps = _prepare_in_maps(keypoints, bev_features)
    res = run_bass_kernel_spmd(nc, in_maps, list(range(N_CORES))).results
    return _assemble(res)



# revision 14
# speedup vs baseline: 1.2158x; 1.2158x over previous
"""Trainium2 Bass kernel: bilinear interpolation from BEV feature maps.

reference semantics (interpolate_from_bev_features, correction=False):
  keypoints (B, N, 3) f32; bev_features (B, C, H, W) f32; bev_stride scalar
  out (B, N, C) f32: bilinear sample at x = kp_x/(0.05*stride),
  y = (kp_y+40)/(0.05*stride); corner indices clamped to [0, 187]; weights
  from clamped corner coords (out-of-range y cancels to exactly 0).

Sharding: 8 cores = batch (4) x channel-half (2).

Per-core design (v2, SBUF-resident bf16 map, no DRAM round trip):
  The map stays CHANNEL-MAJOR in SBUF as bf16 "pair windows": window t =
  [px(2t,y), px(2t+1,y), px(2t,y+1), px(2t+1,y+1)] (8 B per channel,
  2x-duplicated layout, t = y*89 + x/2). An SBUF-source transpose-mode
  dma_gather (tokens_per_rank=1, free_dim_per_rank=8) pulls, per keypoint,
  the 4-pixel window across all 128 channel partitions and transpose-writes
  it channel-major: G[p, q, i] = slot j=p&3 of channel 32q+(p>>2).
  Per keypoint only 2 gathers (x-pair t0 and t0+1); x-parity and the y-clamp
  fold into 8 per-keypoint slot weights.

  Weights are built in wrapped-16 layout on DVE, broadcast to the
  [128-partition, kp-free] layout with one-hot TensorE matmuls (hidden under
  the phase-A DMA loads), applied as a bf16 DVE premultiply, and the 4-slot
  interleave is resolved by TensorE matmuls (M32 one-hot) accumulating into
  keypoint-free PSUM [128ch, 512kp]. Output is stored channel-major
  [128, N] to DRAM (512B+ runs); the host transposes at assembly.

Shapes hardcoded per problem spec: B=4 N=4096 C=256 H=W=188 (x<=176 so
only W_PACK=178 columns are ever addressed). Features cast to bf16 on the
host (rel-err budget 2e-2; bf16 path lands ~3e-3).
"""
import os
import sys

for _p in ('/opt/trn_rl_repo', '/root/.axon_site/_ro/trn_rl_repo'):
    if os.path.isdir(_p) and _p not in sys.path:
        sys.path.append(_p)

import numpy as np

B, N, C, H, W = 4, 4096, 256, 188, 188
W_PACK = 178                  # x <= 176 -> x1 <= 177; cols 178..187 never read
PPR = W_PACK // 2             # 89 x-pairs per row
NPAIR = H * PPR               # 16732 pair ids t = y*89 + (x>>1)
NRANK = 16768                 # TB2 windows = 131 ranks x 128 tokens
NRK = NRANK // 128            # 131 window-ranks per partition
SRC_PX = 33792                # zero-padded compact bf16 map pixels
FLAT = H * W_PACK             # 33464 real pixels
CH = 512                      # keypoints per phase-B chunk
NCH = N // CH                 # 8 chunks
N_CORES = 8

_compiled = {}


def _build(scale: float, ybias: float):
    import concourse.bacc as bacc
    import concourse.mybir as mybir
    import concourse.tile as tile
    import contextlib

    dt = mybir.dt
    nc = bacc.Bacc("TRN2", target_bir_lowering=False, debug=False,
                   num_devices=N_CORES)

    slab_d = nc.dram_tensor("slab", [128, SRC_PX], dt.bfloat16, kind="ExternalInput")
    kpw_d = nc.dram_tensor("kpw", [16, 768], dt.float32, kind="ExternalInput")
    sel_d = nc.dram_tensor("sel", [128, 16 * 128], dt.bfloat16, kind="ExternalInput")
    m32_d = nc.dram_tensor("m32", [128, 32], dt.bfloat16, kind="ExternalInput")
    idn_d = nc.dram_tensor("idn", [128, 128], dt.bfloat16, kind="ExternalInput")
    outT_d = nc.dram_tensor("outT", [128, N], dt.bfloat16, kind="ExternalOutput")

    AF = mybir.ActivationFunctionType
    OP = mybir.AluOpType

    with tile.TileContext(nc) as tc, contextlib.ExitStack() as ctx:
        tbp = ctx.enter_context(tc.tile_pool(name="tbp", bufs=1))
        stg = ctx.enter_context(tc.tile_pool(name="stg", bufs=2))
        meta = ctx.enter_context(tc.tile_pool(name="meta", bufs=1))
        wgt = ctx.enter_context(tc.tile_pool(name="wgt", bufs=1))
        gat = ctx.enter_context(tc.tile_pool(name="gat", bufs=1))
        ops = ctx.enter_context(tc.tile_pool(name="ops", bufs=1))

        # ---- persistent tiles ----
        # TB2[tok, rank, c, k]: window t = rank*128+tok; k = 4 px slots
        TB2 = tbp.tile([128, NRK, 128, 4], dt.bfloat16)
        W0 = wgt.tile([128, N], dt.bfloat16, tag="W0")      # slot weights elem t0
        W1 = wgt.tile([128, N], dt.bfloat16, tag="W1")      # slot weights elem t0+1
        M32 = wgt.tile([128, 32], dt.bfloat16, tag="M32")
        SEL = wgt.tile([128, 16, 128], dt.bfloat16, tag="SEL")
        IDN = wgt.tile([128, 128], dt.bfloat16, tag="IDN")
        nc.scalar.dma_start(M32[:], m32_d.ap())
        nc.scalar.dma_start(SEL[:].rearrange("p r c -> p (r c)"), sel_d.ap())
        nc.scalar.dma_start(IDN[:], idn_d.ap())

        # ---- keypoint math (wrapped-16, replicated to 128 partitions) ----
        KPW = meta.tile([128, 768], dt.float32)
        for g in range(8):
            nc.sync.dma_start(KPW[16 * g:16 * (g + 1), :], kpw_d.ap())
        kp3 = KPW[:].rearrange("p (s c) -> p s c", c=3)

        def floor_of(v_ap, nfree, tag):
            """floor(v) for v >= 0, exact under trunc or round f32<->i32."""
            CI = meta.tile([128, nfree], dt.int32, tag=tag + "i")
            nc.vector.tensor_copy(out=CI[:], in_=v_ap)
            CF = meta.tile([128, nfree], dt.float32, tag=tag + "f")
            nc.vector.tensor_copy(out=CF[:], in_=CI[:])
            GT = meta.tile([128, nfree], dt.float32, tag=tag + "g")
            nc.vector.tensor_tensor(GT[:], CF[:], v_ap, op=OP.is_gt)
            OUT = meta.tile([128, nfree], dt.float32, tag=tag + "o")
            nc.vector.tensor_tensor(OUT[:], CF[:], GT[:], op=OP.subtract)
            return OUT

        XS = meta.tile([128, 256], dt.float32)
        nc.scalar.activation(XS[:], kp3[:, :, 0], AF.Copy, bias=0.0, scale=scale)
        YS = meta.tile([128, 256], dt.float32)
        nc.scalar.activation(YS[:], kp3[:, :, 1], AF.Copy, bias=ybias, scale=scale)

        X0 = floor_of(XS[:], 256, "fx")
        FX = meta.tile([128, 256], dt.float32)
        nc.vector.tensor_tensor(FX[:], XS[:], X0[:], op=OP.subtract)
        WXL = meta.tile([128, 256], dt.float32)
        nc.vector.tensor_scalar(WXL[:], FX[:], 1.0, -1.0, OP.subtract, OP.mult)
        XH = meta.tile([128, 256], dt.float32)
        nc.vector.tensor_scalar(XH[:], X0[:], 0.5, None, OP.mult)
        XP = floor_of(XH[:], 256, "fq")                     # pair col = x0>>1
        EPS = meta.tile([128, 256], dt.float32)             # x0 parity
        nc.vector.tensor_scalar(EPS[:], XP[:], -2.0, None, OP.mult)
        nc.vector.tensor_tensor(EPS[:], X0[:], EPS[:], op=OP.add)
        EPM = meta.tile([128, 256], dt.float32)
        nc.vector.tensor_scalar(EPM[:], EPS[:], 1.0, -1.0, OP.subtract, OP.mult)

        T = floor_of(YS[:], 256, "fy")
        Y0F = meta.tile([128, 256], dt.float32)
        nc.vector.tensor_scalar(Y0F[:], T[:], float(H - 1), None, OP.min)
        Y1F = meta.tile([128, 256], dt.float32)
        nc.vector.tensor_scalar(Y1F[:], T[:], 1.0, float(H - 1), OP.add, OP.min)
        WY0 = meta.tile([128, 256], dt.float32)
        nc.vector.tensor_tensor(WY0[:], Y1F[:], YS[:], op=OP.subtract)
        WY1 = meta.tile([128, 256], dt.float32)
        nc.vector.tensor_tensor(WY1[:], YS[:], Y0F[:], op=OP.subtract)
        # y-clamp (y0==y1): fold wy1 into wy0 (sums to exactly 0), zero wy1
        MEQ = meta.tile([128, 256], dt.float32)
        nc.vector.tensor_tensor(MEQ[:], Y0F[:], Y1F[:], op=OP.is_ge)
        WY1M = meta.tile([128, 256], dt.float32)
        nc.vector.tensor_tensor(WY1M[:], WY1[:], MEQ[:], op=OP.mult)
        WY0P = meta.tile([128, 256], dt.float32)
        nc.vector.tensor_tensor(WY0P[:], WY0[:], WY1M[:], op=OP.add)
        WY1P = meta.tile([128, 256], dt.float32)
        nc.vector.tensor_tensor(WY1P[:], WY1[:], WY1M[:], op=OP.subtract)

        # x-slot factors: A = wxl*(1-eps); Bx = wxl*eps + fx*(1-eps); Cx = fx*eps
        A = meta.tile([128, 256], dt.float32)
        nc.vector.tensor_tensor(A[:], WXL[:], EPM[:], op=OP.mult)
        BX = meta.tile([128, 256], dt.float32)
        T1T = meta.tile([128, 256], dt.float32)
        nc.vector.tensor_tensor(T1T[:], WXL[:], EPS[:], op=OP.mult)
        nc.vector.tensor_tensor(BX[:], FX[:], EPM[:], op=OP.mult)
        nc.vector.tensor_tensor(BX[:], BX[:], T1T[:], op=OP.add)
        CX = meta.tile([128, 256], dt.float32)
        nc.vector.tensor_tensor(CX[:], FX[:], EPS[:], op=OP.mult)

        # gather indices t0 = y0*89 + (x0>>1), t1 = t0 + 1 (int16, wrapped-16)
        T0F = meta.tile([128, 256], dt.float32)
        nc.vector.tensor_scalar(T0F[:], Y0F[:], float(PPR), None, OP.mult)
        nc.vector.tensor_tensor(T0F[:], T0F[:], XP[:], op=OP.add)
        T0 = meta.tile([128, 256], dt.int16)
        nc.vector.tensor_copy(out=T0[:], in_=T0F[:])
        T1F = meta.tile([128, 256], dt.float32)
        nc.vector.tensor_scalar(T1F[:], T0F[:], 1.0, None, OP.add)
        T1 = meta.tile([128, 256], dt.int16)
        nc.vector.tensor_copy(out=T1[:], in_=T1F[:])

        # slot-weight stacks [64, 256]: partition 16j+r = v_{k,j}(s*16+r)
        VS0 = wgt.tile([128, 256], dt.bfloat16, tag="VS0")
        VS1 = wgt.tile([128, 256], dt.bfloat16, tag="VS1")
        for j, (xf, yf) in enumerate(((A, WY0P), (BX, WY0P), (A, WY1P), (BX, WY1P))):
            sl = slice(32 * j, 32 * (j + 1))
            nc.vector.tensor_tensor(VS0[sl, :], xf[sl, :], yf[sl, :], op=OP.mult)
        nc.vector.memset(VS1[:], 0.0)
        for j, yf in ((0, WY0P), (2, WY1P)):
            sl = slice(32 * j, 32 * (j + 1))
            nc.vector.tensor_tensor(VS1[sl, :], CX[sl, :], yf[sl, :], op=OP.mult)

        psB = ctx.enter_context(tc.tile_pool(name="psB", bufs=2, space="PSUM"))

        # ---- phase A: stream slab -> pixel-major interleaved windows ----
        # Window t needs px {2t+kx+178*ky}; PE transposes (matmul vs
        # identity) turn [128ch, 128px] into [128px, 128ch] in PSUM, then
        # strided copies interleave (c,k) into TB2[tok, rank, c, k].
        psA = ctx.enter_context(tc.tile_pool(name="psA", bufs=2, space="PSUM"))
        GRP = 8                                             # ranks per staging
        for g in range((NRK + GRP - 1) // GRP):
            r0, r1 = g * GRP, min((g + 1) * GRP, NRK)
            px0 = r0 * 256
            pxn = min(2 * 128 * (r1 - r0) + 180, SRC_PX - px0)
            STG = stg.tile([128, 2 * 128 * GRP + 180], dt.bfloat16, tag="STG")
            nc.sync.dma_start(STG[:, :pxn], slab_d.ap()[:, px0:px0 + pxn])
            for rk in range(r0, r1):
                P = psA.tile([128, 4, 128], dt.float32, tag="P")
                for k in range(4):
                    ky, kx = k // 2, k % 2
                    base = (rk - r0) * 256 + 178 * ky
                    sl2 = STG[:, base:base + 256] \
                        .rearrange("p (t two) -> p t two", two=2)
                    nc.tensor.matmul(P[:, k, :], sl2[:, :, kx], IDN[:],
                                     start=True, stop=True)
                if rk % 2 == 0:
                    nc.scalar.copy(out=TB2[:, rk, :, :],
                                   in_=P[:].rearrange("p k c -> p c k"))
                else:
                    nc.vector.tensor_copy(out=TB2[:, rk, :, :],
                                          in_=P[:].rearrange("p k c -> p c k"))

        # ---- W build: one-hot matmuls (TensorE) ----
        with tc.tile_pool(name="psW", bufs=1, space="PSUM") as psW:
            for k, (VS, Wk) in enumerate(((VS0, W0), (VS1, W1))):
                for h in range(2):
                    WPS = psW.tile([128, 8, 256], dt.float32, tag="WPS")
                    for r8 in range(8):
                        r = 8 * h + r8
                        nc.tensor.matmul(WPS[:, r8, :], SEL[:, r, :], VS[:],
                                         start=True, stop=True)
                    # W[pi, s*16+r] <- WPS[pi, r8, s]
                    nc.scalar.copy(
                        out=Wk[:].rearrange("p (s r) -> p r s", r=16)
                            [:, 8 * h:8 * h + 8, :],
                        in_=WPS[:])

        # ---- phase B: gather + blend + un-interleave + store ----
        tb_flat = TB2[:].rearrange("p t c k -> p (t c k)")
        for c in range(NCH):
            isl = slice(c * CH, (c + 1) * CH)
            wsl = slice(c * (CH // 16), (c + 1) * (CH // 16))
            GG = []
            for k, (Tk, Wk) in enumerate(((T0, W0), (T1, W1))):
                G = gat.tile([128, 4, CH], dt.bfloat16, tag=f"G{k}")
                nc.gpsimd.dma_gather(
                    out_ap=G[:], in_ap=tb_flat, idxs_ap=Tk[:, wsl],
                    num_idxs=CH, num_idxs_reg=CH, elem_size=512,
                    transpose=True, sbuf_tokens_per_rank=128,
                    sbuf_free_dim_per_rank=1024)
                wb = Wk[:, None, isl].to_broadcast((128, 4, CH))
                nc.vector.tensor_tensor(G[:], G[:], wb, op=OP.mult)
                GG.append(G)
            U = psB.tile([128, CH], dt.float32, tag="U")
            for k, G in enumerate(GG):
                for q in range(4):
                    nc.tensor.matmul(U[32 * q:32 * (q + 1), :], M32[:], G[:, q, :],
                                     start=(k == 0), stop=(k == 1),
                                     tile_position=(0, 32 * q),
                                     skip_group_check=True)
            USB = ops.tile([128, CH], dt.bfloat16, tag="USB")
            nc.scalar.copy(out=USB[:], in_=U[:])
            nc.sync.dma_start(outT_d.ap()[:, isl], USB[:])

    nc.compile()
    return nc


def _get(scale: float, ybias: float):
    key = (round(scale, 9), round(ybias, 9))
    if key not in _compiled:
        _compiled[key] = _build(scale, ybias)
    return _compiled[key]


def _host_consts():
    import ml_dtypes
    # SEL[kappa, r, pi] = [kappa == 16*(pi&3) + r]  (one-hot weight broadcast)
    kap = np.arange(128)[:, None, None]
    r = np.arange(16)[None, :, None]
    pi = np.arange(128)[None, None, :]
    sel = (kap == 32 * (pi % 4) + r).astype(ml_dtypes.bfloat16)
    idn = np.eye(128).astype(ml_dtypes.bfloat16)
    # M32[pi, c'] = [pi>>2 == c']  (4-slot sum + channel un-interleave)
    m32 = ((np.arange(128)[:, None] // 4) == np.arange(32)[None, :]) \
        .astype(ml_dtypes.bfloat16)
    return sel.reshape(128, 16 * 128), m32, idn


def _prepare_in_maps(keypoints: np.ndarray, bev_features: np.ndarray):
    import ml_dtypes
    kp = np.ascontiguousarray(keypoints, dtype=np.float32)
    bev = np.asarray(bev_features, dtype=np.float32)
    sel, m32, idn = _host_consts()
    in_maps = []
    for core in range(N_CORES):
        b, ch = core // 2, core % 2
        sl = slice(ch * 128, (ch + 1) * 128)
        slab = np.zeros((128, SRC_PX), dtype=ml_dtypes.bfloat16)
        slab[:, :FLAT] = bev[b, sl, :, :W_PACK].reshape(128, FLAT) \
            .astype(ml_dtypes.bfloat16)
        kpw = np.ascontiguousarray(
            kp[b].reshape(256, 16, 3).transpose(1, 0, 2)).reshape(16, 768)
        in_maps.append({"slab": slab, "kpw": kpw, "sel": sel, "m32": m32, "idn": idn})
    return in_maps


def _assemble(results) -> np.ndarray:
    out = np.empty((B, N, C), dtype=np.float32)
    for core in range(N_CORES):
        b, ch = core // 2, core % 2
        out[b, :, ch * 128:(ch + 1) * 128] = np.asarray(results[core]["outT"]).astype(np.float32).T
    return out


def _scale_bias(bev_stride):
    stride = float(np.asarray(bev_stride))
    scale = 1.0 / (0.05 * stride)
    return scale, 40.0 * scale


def kernel(keypoints: np.ndarray, bev_features: np.ndarray, bev_stride) -> np.ndarray:
    from concourse.bass_utils import run_bass_kernel_spmd

    scale, ybias = _scale_bias(bev_stride)
    nc = _get(scale, ybias)
    in_maps = _prepare_in_maps(keypoints, bev_features)
    res = run_bass_kernel_spmd(nc, in_maps, list(range(N_CORES))).results
    return _assemble(res)
